# revision 57
# baseline (speedup 1.0000x reference)
"""Trainium2 Bass kernel for Gemma3 sliding-window attention.

Problem: B=1, T=4096, d_model=2048, 8 query heads / 4 KV heads, head_dim=256,
sliding window 1024, per-head RMSNorm + RoPE (interleaved rotate-half with
cat(freqs,freqs) tables), o_proj.

Sharding (8 cores): 4 KV-head groups x 2 sequence halves. Core (g, s) computes
query heads {2g, 2g+1} and KV head g for query tokens [s*2048, (s+1)*2048),
with a 1024-token KV halo (recomputed locally; s=0's halo is zero-padded and
masked out via the exp bias). Each core emits a partial o-projection
[2048, 2048]; the host sums the 4 group partials per half.

v3 dataflow (bf16): host pre-transposes x and all weights so every matmul
operand loads in its natural layout. RoPE is restructured by host-side
column permutation of Wq/Wk (even logical dims in d-subtile 0, odd in
subtile 1) so rotate-half needs no PE permutation matmul - just elementwise
multiplies against 4 per-parity cos/sin tables (bf16), which run on the
otherwise-idle Pool engine. Attention is computed in S.T orientation per
512-query block: S.T[j,i] = kT.T @ qT per 128-j tile, P.T = exp(S.T/16 +
bias) on ACT (bias -1e5 kills invalid j for the padded half), constant
triangle masks on the 8 window-edge tiles (DVE), softmax denominator via
ones-matmul (no max-subtraction: RMSNorm bounds |scores| <= 16),
y.T = v.T @ P.T, then y.T is scaled by the reciprocal denominator and
consumed as lhsT by the o-projection.

Pipeline scheduling (the PE queue is in-order, so anything that waits on a
slow ACT/DVE/Pool producer blocks all later matmuls):
 - next tile's x / rope-table DMAs are prefetched one tile ahead; the
   late-needed wq / tri / wo loads are issued inside tiles 0-1 so they
   never head-of-line-block sooner-needed x tiles on the SP queue;
 - the v projection is emitted after the q norm/rope chains as PE filler;
 - the o-projection of block a is deferred into tile a+3 so its yt/rc
   dependency chain never stalls the PE;
 - output stores are 512-wide bf16 tiles (8-deep staging ring) alternated
   across the two hwdge queues (ACT, SP); the host accumulates the bf16
   partials in f32.
"""

import sys

if "/opt/trn_rl_repo" not in sys.path:
    sys.path.insert(0, "/opt/trn_rl_repo")

import numpy as np

try:
    import ml_dtypes
    BF16 = ml_dtypes.bfloat16
except ImportError:
    BF16 = None

T, DM, NH, NKV, HD, WIN = 4096, 2048, 8, 4, 256, 1024
EPS, BASE = 1e-6, 10000.0
NG, NS = 4, 2
TL, NQ = 3072, 2048
NKO = 16          # 2048 / 128 contraction subtiles
NA = 4            # 512-query attention blocks per core
SCALE = 1.0 / 16.0
NEG = -1.0e5

_cache = {}


def _host_prep(x, pos, Wq, Wk, Wv, Wo, q_norm_w, k_norm_w):
    x = np.asarray(x, np.float32).reshape(T, DM)
    xT = np.ascontiguousarray(x.T)
    pos_f = np.asarray(pos).astype(np.float64)
    m = np.arange(128)
    invf = BASE ** (-m / 128.0)

    Wq = np.asarray(Wq, np.float32)
    Wk = np.asarray(Wk, np.float32)
    Wv = np.asarray(Wv, np.float32)
    Wo = np.asarray(Wo, np.float32)
    qnw = np.asarray(q_norm_w, np.float32)
    knw = np.asarray(k_norm_w, np.float32)

    # deinterleave permutation: even logical dims -> subtile 0, odd -> 1
    perm = np.concatenate([2 * np.arange(128), 2 * np.arange(128) + 1])
    # rope table row indices per parity (freq index = logical dim mod 128)
    epi = (2 * np.arange(128)) % 128
    opi = (2 * np.arange(128) + 1) % 128

    qw2 = np.ascontiguousarray(np.stack([qnw[perm[:128]], qnw[perm[128:]]], axis=1))
    kw2 = np.ascontiguousarray(np.stack([knw[perm[:128]], knw[perm[128:]]], axis=1))

    def permute_heads(W, nheads):
        Wr = W.reshape(nheads, HD, DM)
        return Wr[:, perm, :].reshape(nheads * HD, DM)

    Wqp = permute_heads(Wq, NH)
    Wkp = permute_heads(Wk, NKV)

    ones = np.ones((128, 128), np.float32)

    # masks for 512-wide attention blocks: m=0..3 far edge, m=4..7 diagonal
    jp = np.arange(128)[:, None]
    ip = np.arange(512)[None, :]
    tris = []
    for mm_ in range(4):
        tris.append(jp >= ip + 1 - 128 * mm_)         # far masks F_m
    for mm_ in range(4):
        tris.append(jp <= ip - 128 * mm_)             # diag masks D_{m+8}
    tri = np.concatenate(tris, axis=1).astype(BF16)   # [128, 8*512]

    in_maps = []
    for g in range(NG):
        for s in range(NS):
            lo = s * 2048 - 1024
            xT_c = np.zeros((DM, TL), np.float32)
            src_lo = max(lo, 0)
            xT_c[:, src_lo - lo:] = xT[:, src_lo:(s + 1) * 2048]
            pidx = np.clip(np.arange(lo, lo + TL), 0, T - 1)
            p = pos_f[pidx]
            p[np.arange(lo, lo + TL) < 0] = 0.0
            angE = p[None, :] * invf[epi][:, None]
            angO = p[None, :] * invf[opi][:, None]
            tabs = np.stack([np.cos(angE), np.sin(angE),
                             np.cos(angO), np.sin(angO)], axis=0)  # [4,128,TL]

            kbias = np.zeros((128, 24), np.float32)
            if s == 0:
                kbias[:, :8] = NEG

            in_maps.append({
                "xT": xT_c.astype(BF16),
                "tabs": tabs.astype(BF16),
                "wqT": np.ascontiguousarray(
                    Wqp[2 * g * HD:(2 * g + 2) * HD, :].T).astype(BF16),
                "wkT": np.ascontiguousarray(
                    Wkp[g * HD:(g + 1) * HD, :].T).astype(BF16),
                "wvT": np.ascontiguousarray(
                    Wv[g * HD:(g + 1) * HD, :].T).astype(BF16),
                "woT": np.ascontiguousarray(
                    Wo[:, 2 * g * HD:(2 * g + 2) * HD].T).astype(BF16),
                "ones_bf": ones.astype(BF16),
                "qw": qw2,
                "kw": kw2,
                "kbias": kbias,
                "tri": tri,
            })
    return in_maps


def _build_program():
    if "nc" in _cache:
        return _cache["nc"]

    import concourse.bass as bass
    import concourse.mybir as mybir
    import concourse.tile as tile
    from concourse import bacc
    from contextlib import ExitStack

    f32 = mybir.dt.float32
    bf16 = mybir.dt.bfloat16
    AF = mybir.ActivationFunctionType
    OP = mybir.AluOpType

    nc = bacc.Bacc("TRN2", target_bir_lowering=False, debug=False,
                   enable_asserts=False, num_devices=8)

    xT_d = nc.dram_tensor("xT", [DM, TL], bf16, kind="ExternalInput")
    tabs_d = nc.dram_tensor("tabs", [4, 128, TL], bf16, kind="ExternalInput")
    wq_d = nc.dram_tensor("wqT", [DM, 512], bf16, kind="ExternalInput")
    wk_d = nc.dram_tensor("wkT", [DM, 256], bf16, kind="ExternalInput")
    wv_d = nc.dram_tensor("wvT", [DM, 256], bf16, kind="ExternalInput")
    wo_d = nc.dram_tensor("woT", [512, DM], bf16, kind="ExternalInput")
    onesbf_d = nc.dram_tensor("ones_bf", [128, 128], bf16, kind="ExternalInput")
    qw_d = nc.dram_tensor("qw", [128, 2], f32, kind="ExternalInput")
    kw_d = nc.dram_tensor("kw", [128, 2], f32, kind="ExternalInput")
    kb_d = nc.dram_tensor("kbias", [128, 24], f32, kind="ExternalInput")
    tri_d = nc.dram_tensor("tri", [128, 8 * 512], bf16, kind="ExternalInput")
    o_d = nc.dram_tensor("o_part", [NQ, DM], bf16, kind="ExternalOutput")

    with tile.TileContext(nc) as tc, ExitStack() as ctx:
        cpool = ctx.enter_context(tc.tile_pool(name="consts", bufs=1))
        xpool = ctx.enter_context(tc.tile_pool(name="xt", bufs=4))
        tabpool = ctx.enter_context(tc.tile_pool(name="tab", bufs=2))
        kpool = ctx.enter_context(tc.tile_pool(name="kring", bufs=4))
        vpool = ctx.enter_context(tc.tile_pool(name="vring", bufs=4))
        scpool = ctx.enter_context(tc.tile_pool(name="scratch", bufs=3))
        spool = ctx.enter_context(tc.tile_pool(name="small", bufs=5))
        qpool = ctx.enter_context(tc.tile_pool(name="qt", bufs=4))
        ptpool = ctx.enter_context(tc.tile_pool(name="pt", bufs=5))
        ypool = ctx.enter_context(tc.tile_pool(name="yt", bufs=3))
        opool = ctx.enter_context(tc.tile_pool(name="osb", bufs=8))
        pp_proj = ctx.enter_context(tc.tile_pool(name="pproj", bufs=3, space="PSUM"))
        pp_small = ctx.enter_context(tc.tile_pool(name="psmall", bufs=2, space="PSUM"))
        pp_acc = ctx.enter_context(tc.tile_pool(name="pacc", bufs=3, space="PSUM"))

        # ---- resident constants / weights ----
        # (order matters: the first k matmul needs only wk's first ko slices
        # plus the first half of x tile 0 - chunk those DMAs so PE can start
        # before the full transfers land)
        xT_v0 = xT_d.ap().rearrange("(ko p) t -> p ko t", p=128)
        wk_sb = cpool.tile([128, NKO, 256], bf16, tag="wk")
        nc.sync.dma_start(wk_sb[:, 0:4, :],
                          wk_d.ap().rearrange("(ko p) c -> p ko c", p=128)[:, 0:4, :])
        xt0a = xpool.tile([128, 8, 512], bf16, tag="xt")
        nc.sync.dma_start(xt0a[:, 0:2, :], xT_v0[:, 0:2, 0:512])
        nc.sync.dma_start(xt0a[:, 2:4, :], xT_v0[:, 2:4, 0:512])
        nc.sync.dma_start(wk_sb[:, 4:8, :],
                          wk_d.ap().rearrange("(ko p) c -> p ko c", p=128)[:, 4:8, :])
        nc.sync.dma_start(xt0a[:, 4:8, :], xT_v0[:, 4:8, 0:512])
        nc.sync.dma_start(wk_sb[:, 8:16, :],
                          wk_d.ap().rearrange("(ko p) c -> p ko c", p=128)[:, 8:16, :])
        xt0b = xpool.tile([128, 8, 512], bf16, tag="xt")
        nc.sync.dma_start(xt0b[:, 0:4, :], xT_v0[:, 8:12, 0:512])
        wv_sb = cpool.tile([128, NKO, 256], bf16, tag="wv")
        nc.sync.dma_start(wv_sb[:, 0:8, :],
                          wv_d.ap().rearrange("(ko p) c -> p ko c", p=128)[:, 0:8, :])
        nc.sync.dma_start(xt0b[:, 4:8, :], xT_v0[:, 12:16, 0:512])
        nc.sync.dma_start(wv_sb[:, 8:16, :],
                          wv_d.ap().rearrange("(ko p) c -> p ko c", p=128)[:, 8:16, :])
        pre_x = [xt0a, xt0b]
        ones_sb = cpool.tile([128, 128], bf16, tag="ones")
        nc.sync.dma_start(ones_sb[:], onesbf_d.ap())
        qw_sb = cpool.tile([128, 2], f32, tag="qwt")
        nc.sync.dma_start(qw_sb[:], qw_d.ap())
        kw_sb = cpool.tile([128, 2], f32, tag="kwt")
        nc.sync.dma_start(kw_sb[:], kw_d.ap())
        kb_sb = cpool.tile([128, 24], f32, tag="kb")
        nc.sync.dma_start(kb_sb[:], kb_d.ap())
        from concourse.hw_specs import get_activation_tables
        _tabnames = list(get_activation_tables(nc.m.arch).keys())
        _setid = _tabnames.index("natural_log_exp_and_others")
        nc.scalar.add_instruction(mybir.InstLoadActFuncSet(
            name=nc.get_next_instruction_name(),
            act_func_set_id=_setid, ins=[], outs=[]))
        eps_sb = cpool.tile([128, 1], f32, tag="eps")
        nc.vector.memset(eps_sb[:], EPS)
        zero_sb = cpool.tile([128, 1], f32, tag="zero")
        nc.vector.memset(zero_sb[:], 0.0)
        pre_tab = tabpool.tile([128, 4, 512], bf16, tag="tab")
        nc.sync.dma_start(pre_tab[:],
                          tabs_d.ap().rearrange("f p t -> p f t")[:, :, 0:512])
        # wq/tri/wo are only needed from tb=2 on; their DMAs are issued inside
        # tile 0's body, after the tile-1 prefetch, to avoid head-of-line
        # blocking of sooner-needed x tiles
        wq_sb = cpool.tile([128, NKO, 512], bf16, tag="wq")
        tri_sb = cpool.tile([128, 8 * 512], bf16, tag="tri")
        wo_sb = cpool.tile([128, 4, DM], bf16, tag="wo")

        xT_v = xT_d.ap().rearrange("(ko p) t -> p ko t", p=128)  # [128, 16, TL]
        tabs_v = tabs_d.ap().rearrange("f p t -> p f t")

        NTB = 6                     # 512-token projection tiles
        kt_tiles = [None] * NTB
        vt_tiles = [None] * NTB

        def emit_oproj_chunk(a, yt_sb, msub, dmh, dq):
            # one 512-col chunk of block a's partial o-projection. Deferred
            # one tile from its attention block and interleaved into the
            # next block's j-loop: its matmuls are always ready, so they
            # fill the PE while exp/mask chains resolve. PSUM comes from
            # pp_proj, which is idle during attention.
            c0 = (dmh * 2 + dq) * 512
            o_ps = pp_proj.tile([128, 512], f32, tag="pj")
            for hd in range(4):
                nc.tensor.matmul(o_ps[:],
                                 yt_sb[:, hd, msub * 128:(msub + 1) * 128],
                                 wo_sb[:, hd, c0:c0 + 512],
                                 start=(hd == 0), stop=(hd == 3))
            o_sb = opool.tile([128, 512], bf16, tag="o")
            nc.vector.tensor_copy(o_sb[:], o_ps[:])
            r0_ = a * 512 + msub * 128
            # alternate output stores across both hwdge queues (ACT, SP) so
            # they drain in parallel and never sit behind input prefetches
            eng = nc.scalar if dmh == 0 else nc.sync
            eng.dma_start(o_d.ap()[r0_:r0_ + 128, c0:c0 + 512], o_sb[:])

        OCHUNKS = [(ms, dm, dq) for ms in range(4) for dm in range(2)
                   for dq in range(2)]

        def emit_oproj(a, yt_sb):
            for (ms, dm, dq) in OCHUNKS:
                emit_oproj_chunk(a, yt_sb, ms, dm, dq)

        def norm_rope(src_ps, w_sb, tab, dst, dsti):
            """src_ps: two PSUM [128, 512] tiles (one head's 2 d-subtiles:
            even logical dims, odd logical dims), transposed projection over
            512 tokens. Writes RMSNorm+RoPE (bf16) into dst[:, dsti+u, :]."""
            z2 = scpool.tile([128, 2, 512], bf16, tag="z2")
            for u in range(2):
                nc.scalar.activation(z2[:, u, :], src_ps[u][:], AF.Square,
                                     bias=zero_sb[:])
            ssq = pp_small.tile([128, 512], f32, tag="psm")
            for u in range(2):
                nc.tensor.matmul(ssq[:], ones_sb[:], z2[:, u, :],
                                 start=(u == 0), stop=(u == 1))
            lnt = spool.tile([128, 512], f32, tag="lnt")
            nc.scalar.activation(lnt[:], ssq[:], AF.Ln, bias=eps_sb[:], scale=1.0 / HD)
            rs = spool.tile([128, 512], f32, tag="rs")
            nc.scalar.activation(rs[:], lnt[:], AF.Exp, bias=zero_sb[:], scale=-0.5)
            znw = scpool.tile([128, 2, 512], bf16, tag="znw")
            for u in range(2):
                nc.vector.scalar_tensor_tensor(
                    znw[:, u, :], src_ps[u][:], w_sb[:, u:u + 1], rs[:],
                    OP.mult, OP.mult)
            # rope: out_e = z_e*cosE - z_o*sinE ; out_o = z_o*cosO + z_e*sinO
            # (Pool engine handles the SBUF-only multiplies)
            t1 = spool.tile([128, 512], bf16, tag="t1")
            nc.gpsimd.tensor_tensor(t1[:], znw[:, 0, :], tab[:, 0, :], OP.mult)
            t2 = spool.tile([128, 512], bf16, tag="t2")
            nc.gpsimd.tensor_tensor(t2[:], znw[:, 1, :], tab[:, 1, :], OP.mult)
            nc.gpsimd.tensor_tensor(dst[:, dsti + 0, :], t1[:], t2[:],
                                    OP.subtract)
            t3 = spool.tile([128, 512], bf16, tag="t3")
            nc.gpsimd.tensor_tensor(t3[:], znw[:, 1, :], tab[:, 2, :], OP.mult)
            t4 = spool.tile([128, 512], bf16, tag="t4")
            nc.gpsimd.tensor_tensor(t4[:], znw[:, 0, :], tab[:, 3, :], OP.mult)
            nc.gpsimd.tensor_tensor(dst[:, dsti + 1, :], t3[:], t4[:], OP.add)

        cur_x, cur_tab = pre_x, pre_tab
        yt_prev, a_prev = None, None
        for tb in range(NTB):
            xth = cur_x
            tab = cur_tab
            # prefetch next tile's x and rope tables while this tile computes
            if tb + 1 < NTB:
                t1_ = (tb + 1) * 512
                nxt = []
                for half in range(2):
                    xt = xpool.tile([128, 8, 512], bf16, tag="xt")
                    nc.sync.dma_start(xt[:], xT_v[:, half * 8:(half + 1) * 8, t1_:t1_ + 512])
                    nxt.append(xt)
                ntab = tabpool.tile([128, 4, 512], bf16, tag="tab")
                nc.sync.dma_start(ntab[:], tabs_v[:, :, t1_:t1_ + 512])
                cur_x, cur_tab = nxt, ntab
            if tb == 0:
                nc.sync.dma_start(wq_sb[:],
                                  wq_d.ap().rearrange("(ko p) c -> p ko c", p=128))
            elif tb == 1:
                nc.sync.dma_start(tri_sb[:], tri_d.ap())
                nc.sync.dma_start(wo_sb[:],
                                  wo_d.ap().rearrange("(hd p) c -> p hd c", p=128))

            # ---- k projection (transposed, N=512) ----
            k0_ps = pp_proj.tile([128, 512], f32, tag="pj")
            k1_ps = pp_proj.tile([128, 512], f32, tag="pj")
            k_ps = [k0_ps, k1_ps]
            for dsub in range(2):
                for ko in range(NKO):
                    nc.tensor.matmul(k_ps[dsub][:],
                                     wk_sb[:, ko, dsub * 128:(dsub + 1) * 128],
                                     xth[ko // 8][:, ko % 8, :],
                                     start=(ko == 0), stop=(ko == NKO - 1))
            kt = kpool.tile([128, 2, 512], bf16, tag="kt")
            norm_rope(k_ps, kw_sb, tab, kt, 0)
            kt_tiles[tb] = kt

            # ---- q projections (2 heads, N=512; per-head tiles keep the
            # h0 score matmuls independent of h1's norm/rope chain) ----
            if tb >= 2:
                qt_h0 = qpool.tile([128, 2, 512], bf16, tag="q")
                qt_h1 = qpool.tile([128, 2, 512], bf16, tag="q")
                qt_hs = [qt_h0, qt_h1]
                for h in range(2):
                    q0_ps = pp_proj.tile([128, 512], f32, tag="pj")
                    q1_ps = pp_proj.tile([128, 512], f32, tag="pj")
                    q_ps = [q0_ps, q1_ps]
                    for u in range(2):
                        dsub = 2 * h + u
                        for ko in range(NKO):
                            nc.tensor.matmul(q_ps[u][:],
                                             wq_sb[:, ko, dsub * 128:(dsub + 1) * 128],
                                             xth[ko // 8][:, ko % 8, :],
                                             start=(ko == 0), stop=(ko == NKO - 1))
                    norm_rope(q_ps, qw_sb, tab, qt_hs[h], 0)

            # ---- v projection: emitted after the q norms so its matmuls
            # fill the PE queue while the norm/rope chains complete ----
            vt = vpool.tile([128, 4, 256], bf16, tag="vt")
            for vh in range(2):
                v_ps = pp_proj.tile([128, 2, 256], f32, tag="pj")
                for ms in range(2):
                    msub = vh * 2 + ms
                    for ko in range(NKO):
                        nc.tensor.matmul(v_ps[:, ms, :],
                                         xth[ko // 8][:, ko % 8, msub * 128:(msub + 1) * 128],
                                         wv_sb[:, ko, :],
                                         start=(ko == 0), stop=(ko == NKO - 1))
                for ms in range(2):
                    nc.scalar.copy(vt[:, vh * 2 + ms, :], v_ps[:, ms, :])
            vt_tiles[tb] = vt

            if tb < 2:
                continue

            # ---- attention for 512-query block a ----
            a = tb - 2
            oc = list(OCHUNKS) if yt_prev is not None else []
            oci = 0
            yt_sb = ypool.tile([128, 4, 512], bf16, tag="y")
            for h in range(2):
                dn_ps = pp_acc.tile([128, 512], f32, tag="pac")
                y0_ps = pp_acc.tile([128, 512], f32, tag="pac")
                y1_ps = pp_acc.tile([128, 512], f32, tag="pac")
                y_ps = [y0_ps, y1_ps]
                for mi, mrel in enumerate([3, 0, 1, 2] + list(range(4, 12))):
                    jt = 4 * a + mrel
                    ct, jh = jt // 4, jt % 4
                    ktc = kt_tiles[ct]
                    vtc = vt_tiles[ct]
                    # active query range: edge tiles are mostly masked
                    if mrel <= 2:
                        ia, ib = 0, 128 * (mrel + 1)
                    elif mrel >= 9:
                        ia, ib = 128 * (mrel - 8), 512
                    else:
                        ia, ib = 0, 512
                    pt = ptpool.tile([128, 512], bf16, tag="p")
                    st = pp_small.tile([128, 512], f32, tag="psm")
                    for u in range(2):
                        nc.tensor.matmul(st[:, ia:ib],
                                         ktc[:, u, jh * 128:(jh + 1) * 128],
                                         qt_hs[h][:, u, ia:ib],
                                         start=(u == 0), stop=(u == 1))
                    nc.scalar.activation(pt[:, ia:ib], st[:, ia:ib], AF.Exp,
                                         bias=kb_sb[:, jt:jt + 1], scale=SCALE)
                    if mrel < 4:
                        nc.vector.tensor_tensor(
                            pt[:, ia:ib], pt[:, ia:ib],
                            tri_sb[:, mrel * 512 + ia:mrel * 512 + ib], OP.mult)
                    elif mrel >= 8:
                        nc.vector.tensor_tensor(
                            pt[:, ia:ib], pt[:, ia:ib],
                            tri_sb[:, (mrel - 4) * 512 + ia:(mrel - 4) * 512 + ib],
                            OP.mult)
                    first, last = (mi == 0), (mrel == 11)
                    nc.tensor.matmul(dn_ps[:, ia:ib], ones_sb[:], pt[:, ia:ib],
                                     start=first, stop=last, skip_group_check=True)
                    for dh in range(2):
                        nc.tensor.matmul(y_ps[dh][:, ia:ib],
                                         vtc[:, jh, dh * 128:(dh + 1) * 128],
                                         pt[:, ia:ib], start=first, stop=last,
                                         skip_group_check=True)
                    if oci < len(oc) and mi < 8:
                        ms, dm, dq = oc[oci]
                        emit_oproj_chunk(a_prev, yt_prev, ms, dm, dq)
                        oci += 1
                rc = spool.tile([128, 512], f32, tag="rc")
                nc.vector.reciprocal_approx_fast(rc[:], dn_ps[:])
                for dh in range(2):
                    nc.vector.tensor_tensor(yt_sb[:, 2 * h + dh, :],
                                            y_ps[dh][:], rc[:], OP.mult)

            while oci < len(oc):
                ms, dm, dq = oc[oci]
                emit_oproj_chunk(a_prev, yt_prev, ms, dm, dq)
                oci += 1
            yt_prev, a_prev = yt_sb, a

        emit_oproj(a_prev, yt_prev)

    nc.compile()
    _cache["nc"] = nc
    return nc


def _run(inputs, trace=False):
    from concourse.bass_utils import run_bass_kernel_spmd

    nc = _build_program()
    in_maps = _host_prep(**inputs)
    res = run_bass_kernel_spmd(nc, in_maps, core_ids=list(range(8)), trace=trace)
    full = np.zeros((T, DM), np.float32)
    for g in range(NG):
        for s in range(NS):
            full[s * 2048:(s + 1) * 2048] += np.asarray(
                res.results[g * 2 + s]["o_part"], dtype=np.float32)
    return full.reshape(1, T, DM), res


def kernel(**inputs):
    return _run(inputs, trace=False)[0]


# revision 63
# speedup vs baseline: 1.0092x; 1.0092x over previous
"""Trainium2 Bass kernel for Gemma3 sliding-window attention.

Problem: B=1, T=4096, d_model=2048, 8 query heads / 4 KV heads, head_dim=256,
sliding window 1024, per-head RMSNorm + RoPE (interleaved rotate-half with
cat(freqs,freqs) tables), o_proj.

Sharding (8 cores): 4 KV-head groups x 2 sequence halves. Core (g, s) computes
query heads {2g, 2g+1} and KV head g for query tokens [s*2048, (s+1)*2048),
with a 1024-token KV halo (recomputed locally; s=0's halo is zero-padded and
masked out via the exp bias). Each core emits a partial o-projection
[2048, 2048]; the host sums the 4 group partials per half.

v3 dataflow (bf16): host pre-transposes x and all weights so every matmul
operand loads in its natural layout. RoPE is restructured by host-side
column permutation of Wq/Wk (even logical dims in d-subtile 0, odd in
subtile 1) so rotate-half needs no PE permutation matmul - just elementwise
multiplies against 4 per-parity cos/sin tables (bf16), which run on the
otherwise-idle Pool engine. Attention is computed in S.T orientation per
512-query block: S.T[j,i] = kT.T @ qT per 128-j tile, P.T = exp(S.T/16 +
bias) on ACT (bias -1e5 kills invalid j for the padded half), constant
triangle masks on the 8 window-edge tiles (DVE), softmax denominator via
ones-matmul (no max-subtraction: RMSNorm bounds |scores| <= 16),
y.T = v.T @ P.T, then y.T is scaled by the reciprocal denominator and
consumed as lhsT by the o-projection.

Pipeline scheduling (the PE queue is in-order, so anything that waits on a
slow ACT/DVE/Pool producer blocks all later matmuls):
 - next tile's x / rope-table DMAs are prefetched one tile ahead; the
   late-needed wq / tri / wo loads are issued inside tiles 0-1 so they
   never head-of-line-block sooner-needed x tiles on the SP queue;
 - the v projection is emitted after the q norm/rope chains as PE filler;
 - the o-projection of block a is deferred into tile a+3 so its yt/rc
   dependency chain never stalls the PE;
 - output stores are 512-wide bf16 tiles (8-deep staging ring) alternated
   across the two hwdge queues (ACT, SP); the host accumulates the bf16
   partials in f32.
"""

import sys

if "/opt/trn_rl_repo" not in sys.path:
    sys.path.insert(0, "/opt/trn_rl_repo")

import numpy as np

try:
    import ml_dtypes
    BF16 = ml_dtypes.bfloat16
except ImportError:
    BF16 = None

T, DM, NH, NKV, HD, WIN = 4096, 2048, 8, 4, 256, 1024
EPS, BASE = 1e-6, 10000.0
NG, NS = 4, 2
TL, NQ = 3072, 2048
NKO = 16          # 2048 / 128 contraction subtiles
NA = 4            # 512-query attention blocks per core
SCALE = 1.0 / 16.0
NEG = -1.0e5

_cache = {}


def _host_prep(x, pos, Wq, Wk, Wv, Wo, q_norm_w, k_norm_w):
    x = np.asarray(x, np.float32).reshape(T, DM)
    xT = np.ascontiguousarray(x.T)
    pos_f = np.asarray(pos).astype(np.float64)
    m = np.arange(128)
    invf = BASE ** (-m / 128.0)

    Wq = np.asarray(Wq, np.float32)
    Wk = np.asarray(Wk, np.float32)
    Wv = np.asarray(Wv, np.float32)
    Wo = np.asarray(Wo, np.float32)
    qnw = np.asarray(q_norm_w, np.float32)
    knw = np.asarray(k_norm_w, np.float32)

    # deinterleave permutation: even logical dims -> subtile 0, odd -> 1
    perm = np.concatenate([2 * np.arange(128), 2 * np.arange(128) + 1])
    # rope table row indices per parity (freq index = logical dim mod 128)
    epi = (2 * np.arange(128)) % 128
    opi = (2 * np.arange(128) + 1) % 128

    qw2 = np.ascontiguousarray(np.stack([qnw[perm[:128]], qnw[perm[128:]]], axis=1))
    kw2 = np.ascontiguousarray(np.stack([knw[perm[:128]], knw[perm[128:]]], axis=1))

    def permute_heads(W, nheads):
        Wr = W.reshape(nheads, HD, DM)
        return Wr[:, perm, :].reshape(nheads * HD, DM)

    Wqp = permute_heads(Wq, NH)
    Wkp = permute_heads(Wk, NKV)

    ones = np.ones((128, 128), np.float32)

    # masks for 512-wide attention blocks: m=0..3 far edge, m=4..7 diagonal
    jp = np.arange(128)[:, None]
    ip = np.arange(512)[None, :]
    tris = []
    for mm_ in range(4):
        tris.append(jp >= ip + 1 - 128 * mm_)         # far masks F_m
    for mm_ in range(4):
        tris.append(jp <= ip - 128 * mm_)             # diag masks D_{m+8}
    tri = np.concatenate(tris, axis=1).astype(BF16)   # [128, 8*512]

    in_maps = []
    for g in range(NG):
        for s in range(NS):
            lo = s * 2048 - 1024
            xT_c = np.zeros((DM, TL), np.float32)
            src_lo = max(lo, 0)
            xT_c[:, src_lo - lo:] = xT[:, src_lo:(s + 1) * 2048]
            pidx = np.clip(np.arange(lo, lo + TL), 0, T - 1)
            p = pos_f[pidx]
            p[np.arange(lo, lo + TL) < 0] = 0.0
            angE = p[None, :] * invf[epi][:, None]
            angO = p[None, :] * invf[opi][:, None]
            tabs = np.stack([np.cos(angE), np.sin(angE),
                             np.cos(angO), np.sin(angO)], axis=0)  # [4,128,TL]

            kbias = np.zeros((128, 24), np.float32)
            if s == 0:
                kbias[:, :8] = NEG

            in_maps.append({
                "xT": xT_c.astype(BF16),
                "tabs": tabs.astype(BF16),
                "wqT": np.ascontiguousarray(
                    Wqp[2 * g * HD:(2 * g + 2) * HD, :].T).astype(BF16),
                "wkT": np.ascontiguousarray(
                    Wkp[g * HD:(g + 1) * HD, :].T).astype(BF16),
                "wvT": np.ascontiguousarray(
                    Wv[g * HD:(g + 1) * HD, :].T).astype(BF16),
                "woT": np.ascontiguousarray(
                    Wo[:, 2 * g * HD:(2 * g + 2) * HD].T).astype(BF16),
                "ones_bf": ones.astype(BF16),
                "qw": qw2,
                "kw": kw2,
                "kbias": kbias,
                "tri": tri,
            })
    return in_maps


def _build_program():
    if "nc" in _cache:
        return _cache["nc"]

    import concourse.bass as bass
    import concourse.mybir as mybir
    import concourse.tile as tile
    from concourse import bacc
    from contextlib import ExitStack

    f32 = mybir.dt.float32
    bf16 = mybir.dt.bfloat16
    AF = mybir.ActivationFunctionType
    OP = mybir.AluOpType

    nc = bacc.Bacc("TRN2", target_bir_lowering=False, debug=False,
                   enable_asserts=False, num_devices=8)

    xT_d = nc.dram_tensor("xT", [DM, TL], bf16, kind="ExternalInput")
    tabs_d = nc.dram_tensor("tabs", [4, 128, TL], bf16, kind="ExternalInput")
    wq_d = nc.dram_tensor("wqT", [DM, 512], bf16, kind="ExternalInput")
    wk_d = nc.dram_tensor("wkT", [DM, 256], bf16, kind="ExternalInput")
    wv_d = nc.dram_tensor("wvT", [DM, 256], bf16, kind="ExternalInput")
    wo_d = nc.dram_tensor("woT", [512, DM], bf16, kind="ExternalInput")
    onesbf_d = nc.dram_tensor("ones_bf", [128, 128], bf16, kind="ExternalInput")
    qw_d = nc.dram_tensor("qw", [128, 2], f32, kind="ExternalInput")
    kw_d = nc.dram_tensor("kw", [128, 2], f32, kind="ExternalInput")
    kb_d = nc.dram_tensor("kbias", [128, 24], f32, kind="ExternalInput")
    tri_d = nc.dram_tensor("tri", [128, 8 * 512], bf16, kind="ExternalInput")
    o_d = nc.dram_tensor("o_part", [NQ, DM], bf16, kind="ExternalOutput")

    with tile.TileContext(nc) as tc, ExitStack() as ctx:
        cpool = ctx.enter_context(tc.tile_pool(name="consts", bufs=1))
        xpool = ctx.enter_context(tc.tile_pool(name="xt", bufs=4))
        tabpool = ctx.enter_context(tc.tile_pool(name="tab", bufs=2))
        kpool = ctx.enter_context(tc.tile_pool(name="kring", bufs=4))
        vpool = ctx.enter_context(tc.tile_pool(name="vring", bufs=4))
        scpool = ctx.enter_context(tc.tile_pool(name="scratch", bufs=3))
        spool = ctx.enter_context(tc.tile_pool(name="small", bufs=5))
        qpool = ctx.enter_context(tc.tile_pool(name="qt", bufs=4))
        ptpool = ctx.enter_context(tc.tile_pool(name="pt", bufs=5))
        ypool = ctx.enter_context(tc.tile_pool(name="yt", bufs=3))
        opool = ctx.enter_context(tc.tile_pool(name="osb", bufs=8))
        pp_proj = ctx.enter_context(tc.tile_pool(name="pproj", bufs=3, space="PSUM"))
        pp_small = ctx.enter_context(tc.tile_pool(name="psmall", bufs=2, space="PSUM"))
        pp_acc = ctx.enter_context(tc.tile_pool(name="pacc", bufs=3, space="PSUM"))

        # ---- resident constants / weights ----
        # (order matters: the first k matmul needs only wk's first ko slices
        # plus the first half of x tile 0 - chunk those DMAs so PE can start
        # before the full transfers land)
        xT_v0 = xT_d.ap().rearrange("(ko p) t -> p ko t", p=128)
        wk_sb = cpool.tile([128, NKO, 256], bf16, tag="wk")
        nc.sync.dma_start(wk_sb[:, 0:4, :],
                          wk_d.ap().rearrange("(ko p) c -> p ko c", p=128)[:, 0:4, :])
        xt0a = xpool.tile([128, 8, 512], bf16, tag="xt")
        nc.sync.dma_start(xt0a[:, 0:2, :], xT_v0[:, 0:2, 0:512])
        nc.sync.dma_start(xt0a[:, 2:4, :], xT_v0[:, 2:4, 0:512])
        nc.sync.dma_start(wk_sb[:, 4:8, :],
                          wk_d.ap().rearrange("(ko p) c -> p ko c", p=128)[:, 4:8, :])
        nc.sync.dma_start(xt0a[:, 4:8, :], xT_v0[:, 4:8, 0:512])
        nc.sync.dma_start(wk_sb[:, 8:16, :],
                          wk_d.ap().rearrange("(ko p) c -> p ko c", p=128)[:, 8:16, :])
        xt0b = xpool.tile([128, 8, 512], bf16, tag="xt")
        nc.sync.dma_start(xt0b[:, 0:4, :], xT_v0[:, 8:12, 0:512])
        wv_sb = cpool.tile([128, NKO, 256], bf16, tag="wv")
        nc.sync.dma_start(wv_sb[:, 0:8, :],
                          wv_d.ap().rearrange("(ko p) c -> p ko c", p=128)[:, 0:8, :])
        nc.sync.dma_start(xt0b[:, 4:8, :], xT_v0[:, 12:16, 0:512])
        nc.sync.dma_start(wv_sb[:, 8:16, :],
                          wv_d.ap().rearrange("(ko p) c -> p ko c", p=128)[:, 8:16, :])
        pre_x = [xt0a, xt0b]
        ones_sb = cpool.tile([128, 128], bf16, tag="ones")
        nc.sync.dma_start(ones_sb[:], onesbf_d.ap())
        qw_sb = cpool.tile([128, 2], f32, tag="qwt")
        nc.sync.dma_start(qw_sb[:], qw_d.ap())
        kw_sb = cpool.tile([128, 2], f32, tag="kwt")
        nc.sync.dma_start(kw_sb[:], kw_d.ap())
        kb_sb = cpool.tile([128, 24], f32, tag="kb")
        nc.sync.dma_start(kb_sb[:], kb_d.ap())
        from concourse.hw_specs import get_activation_tables
        _tabnames = list(get_activation_tables(nc.m.arch).keys())
        _setid = _tabnames.index("natural_log_exp_and_others")
        nc.scalar.add_instruction(mybir.InstLoadActFuncSet(
            name=nc.get_next_instruction_name(),
            act_func_set_id=_setid, ins=[], outs=[]))
        eps_sb = cpool.tile([128, 1], f32, tag="eps")
        nc.vector.memset(eps_sb[:], EPS)
        zero_sb = cpool.tile([128, 1], f32, tag="zero")
        nc.vector.memset(zero_sb[:], 0.0)
        pre_tab = tabpool.tile([128, 4, 512], bf16, tag="tab")
        nc.sync.dma_start(pre_tab[:],
                          tabs_d.ap().rearrange("f p t -> p f t")[:, :, 0:512])
        # wq/tri/wo are only needed from tb=2 on; their DMAs are issued inside
        # tile 0's body, after the tile-1 prefetch, to avoid head-of-line
        # blocking of sooner-needed x tiles
        wq_sb = cpool.tile([128, NKO, 512], bf16, tag="wq")
        tri_sb = cpool.tile([128, 8 * 512], bf16, tag="tri")
        wo_sb = cpool.tile([128, 4, DM], bf16, tag="wo")

        xT_v = xT_d.ap().rearrange("(ko p) t -> p ko t", p=128)  # [128, 16, TL]
        tabs_v = tabs_d.ap().rearrange("f p t -> p f t")

        NTB = 6                     # 512-token projection tiles
        kt_tiles = [None] * NTB
        vt_tiles = [None] * NTB

        def emit_oproj_chunk(a, yt_sb, msub, dmh, dq):
            # one 512-col chunk of block a's partial o-projection. Deferred
            # one tile from its attention block and interleaved into the
            # next block's j-loop: its matmuls are always ready, so they
            # fill the PE while exp/mask chains resolve. PSUM comes from
            # pp_proj, which is idle during attention.
            c0 = (dmh * 2 + dq) * 512
            o_ps = pp_proj.tile([128, 512], f32, tag="pj")
            for hd in range(4):
                nc.tensor.matmul(o_ps[:],
                                 yt_sb[:, hd, msub * 128:(msub + 1) * 128],
                                 wo_sb[:, hd, c0:c0 + 512],
                                 start=(hd == 0), stop=(hd == 3))
            o_sb = opool.tile([128, 512], bf16, tag="o")
            nc.vector.tensor_copy(o_sb[:], o_ps[:])
            r0_ = a * 512 + msub * 128
            # alternate output stores across both hwdge queues (ACT, SP) so
            # they drain in parallel and never sit behind input prefetches
            eng = nc.scalar if dmh == 0 else nc.sync
            eng.dma_start(o_d.ap()[r0_:r0_ + 128, c0:c0 + 512], o_sb[:])

        OCHUNKS = [(ms, dm, dq) for ms in range(4) for dm in range(2)
                   for dq in range(2)]

        def emit_oproj(a, yt_sb):
            for (ms, dm, dq) in OCHUNKS:
                emit_oproj_chunk(a, yt_sb, ms, dm, dq)

        def norm_rope(src_ps, w_sb, tab, dst, dsti):
            """src_ps: two PSUM [128, 512] tiles (one head's 2 d-subtiles:
            even logical dims, odd logical dims), transposed projection over
            512 tokens. Writes RMSNorm+RoPE (bf16) into dst[:, dsti+u, :]."""
            z2 = scpool.tile([128, 2, 512], bf16, tag="z2")
            for u in range(2):
                nc.scalar.activation(z2[:, u, :], src_ps[u][:], AF.Square,
                                     bias=zero_sb[:])
            ssq = pp_small.tile([128, 512], f32, tag="psm")
            for u in range(2):
                nc.tensor.matmul(ssq[:], ones_sb[:], z2[:, u, :],
                                 start=(u == 0), stop=(u == 1))
            lnt = spool.tile([128, 512], f32, tag="lnt")
            nc.scalar.activation(lnt[:], ssq[:], AF.Ln, bias=eps_sb[:], scale=1.0 / HD)
            rs = spool.tile([128, 512], f32, tag="rs")
            nc.scalar.activation(rs[:], lnt[:], AF.Exp, bias=zero_sb[:], scale=-0.5)
            znw = scpool.tile([128, 2, 512], bf16, tag="znw")
            for u in range(2):
                nc.vector.scalar_tensor_tensor(
                    znw[:, u, :], src_ps[u][:], w_sb[:, u:u + 1], rs[:],
                    OP.mult, OP.mult)
            # rope: out_e = z_e*cosE - z_o*sinE ; out_o = z_o*cosO + z_e*sinO
            # (Pool engine handles the SBUF-only multiplies)
            t1 = spool.tile([128, 512], bf16, tag="t1")
            nc.gpsimd.tensor_tensor(t1[:], znw[:, 0, :], tab[:, 0, :], OP.mult)
            t2 = spool.tile([128, 512], bf16, tag="t2")
            nc.gpsimd.tensor_tensor(t2[:], znw[:, 1, :], tab[:, 1, :], OP.mult)
            nc.gpsimd.tensor_tensor(dst[:, dsti + 0, :], t1[:], t2[:],
                                    OP.subtract)
            t3 = spool.tile([128, 512], bf16, tag="t3")
            nc.gpsimd.tensor_tensor(t3[:], znw[:, 1, :], tab[:, 2, :], OP.mult)
            t4 = spool.tile([128, 512], bf16, tag="t4")
            nc.gpsimd.tensor_tensor(t4[:], znw[:, 0, :], tab[:, 3, :], OP.mult)
            nc.gpsimd.tensor_tensor(dst[:, dsti + 1, :], t3[:], t4[:], OP.add)

        cur_x, cur_tab = pre_x, pre_tab
        yt_prev, a_prev = None, None
        for tb in range(NTB):
            xth = cur_x
            tab = cur_tab
            # prefetch next tile's x and rope tables while this tile computes
            if tb + 1 < NTB:
                t1_ = (tb + 1) * 512
                nxt = []
                for half in range(2):
                    xt = xpool.tile([128, 8, 512], bf16, tag="xt")
                    nc.sync.dma_start(xt[:], xT_v[:, half * 8:(half + 1) * 8, t1_:t1_ + 512])
                    nxt.append(xt)
                ntab = tabpool.tile([128, 4, 512], bf16, tag="tab")
                nc.sync.dma_start(ntab[:], tabs_v[:, :, t1_:t1_ + 512])
                cur_x, cur_tab = nxt, ntab
            if tb == 0:
                nc.sync.dma_start(wq_sb[:],
                                  wq_d.ap().rearrange("(ko p) c -> p ko c", p=128))
            elif tb == 1:
                nc.sync.dma_start(tri_sb[:], tri_d.ap())
                nc.sync.dma_start(wo_sb[:],
                                  wo_d.ap().rearrange("(hd p) c -> p hd c", p=128))

            # ---- k projection (transposed, N=512) ----
            k0_ps = pp_proj.tile([128, 512], f32, tag="pj")
            k1_ps = pp_proj.tile([128, 512], f32, tag="pj")
            k_ps = [k0_ps, k1_ps]
            for dsub in range(2):
                for ko in range(NKO):
                    nc.tensor.matmul(k_ps[dsub][:],
                                     wk_sb[:, ko, dsub * 128:(dsub + 1) * 128],
                                     xth[ko // 8][:, ko % 8, :],
                                     start=(ko == 0), stop=(ko == NKO - 1))
            kt = kpool.tile([128, 2, 512], bf16, tag="kt")
            norm_rope(k_ps, kw_sb, tab, kt, 0)
            kt_tiles[tb] = kt

            # ---- q projections (2 heads, N=512; per-head tiles keep the
            # h0 score matmuls independent of h1's norm/rope chain) ----
            if tb >= 2:
                qt_h0 = qpool.tile([128, 2, 512], bf16, tag="q")
                qt_h1 = qpool.tile([128, 2, 512], bf16, tag="q")
                qt_hs = [qt_h0, qt_h1]
                for h in range(2):
                    q0_ps = pp_proj.tile([128, 512], f32, tag="pj")
                    q1_ps = pp_proj.tile([128, 512], f32, tag="pj")
                    q_ps = [q0_ps, q1_ps]
                    for u in range(2):
                        dsub = 2 * h + u
                        for ko in range(NKO):
                            nc.tensor.matmul(q_ps[u][:],
                                             wq_sb[:, ko, dsub * 128:(dsub + 1) * 128],
                                             xth[ko // 8][:, ko % 8, :],
                                             start=(ko == 0), stop=(ko == NKO - 1))
                    norm_rope(q_ps, qw_sb, tab, qt_hs[h], 0)

            # ---- v projection: emitted after the q norms so its matmuls
            # fill the PE queue while the norm/rope chains complete ----
            vt = vpool.tile([128, 4, 256], bf16, tag="vt")
            for vh in range(2):
                v_ps = pp_proj.tile([128, 2, 256], f32, tag="pj")
                for ms in range(2):
                    msub = vh * 2 + ms
                    for ko in range(NKO):
                        nc.tensor.matmul(v_ps[:, ms, :],
                                         xth[ko // 8][:, ko % 8, msub * 128:(msub + 1) * 128],
                                         wv_sb[:, ko, :],
                                         start=(ko == 0), stop=(ko == NKO - 1))
                for ms in range(2):
                    nc.scalar.copy(vt[:, vh * 2 + ms, :], v_ps[:, ms, :])
            vt_tiles[tb] = vt

            if tb < 2:
                continue

            # ---- attention for 512-query block a ----
            a = tb - 2
            oc = list(OCHUNKS) if yt_prev is not None else []
            oci = 0
            yt_sb = ypool.tile([128, 4, 512], bf16, tag="y")
            for h in range(2):
                dn_ps = pp_acc.tile([128, 512], f32, tag="pac")
                y0_ps = pp_acc.tile([128, 512], f32, tag="pac")
                y1_ps = pp_acc.tile([128, 512], f32, tag="pac")
                y_ps = [y0_ps, y1_ps]
                for mi, mrel in enumerate([3, 0, 1, 2] + list(range(4, 12))):
                    jt = 4 * a + mrel
                    ct, jh = jt // 4, jt % 4
                    ktc = kt_tiles[ct]
                    vtc = vt_tiles[ct]
                    # active query range: edge tiles are mostly masked
                    if mrel <= 2:
                        ia, ib = 0, 128 * (mrel + 1)
                    elif mrel >= 9:
                        ia, ib = 128 * (mrel - 8), 512
                    else:
                        ia, ib = 0, 512
                    pt = ptpool.tile([128, 512], bf16, tag="p")
                    st = pp_small.tile([128, 512], f32, tag="psm")
                    for u in range(2):
                        nc.tensor.matmul(st[:, ia:ib],
                                         ktc[:, u, jh * 128:(jh + 1) * 128],
                                         qt_hs[h][:, u, ia:ib],
                                         start=(u == 0), stop=(u == 1))
                    nc.scalar.activation(pt[:, ia:ib], st[:, ia:ib], AF.Exp,
                                         bias=kb_sb[:, jt:jt + 1], scale=SCALE)
                    if mrel < 4:
                        nc.vector.tensor_tensor(
                            pt[:, ia:ib], pt[:, ia:ib],
                            tri_sb[:, mrel * 512 + ia:mrel * 512 + ib], OP.mult)
                    elif mrel >= 8:
                        nc.vector.tensor_tensor(
                            pt[:, ia:ib], pt[:, ia:ib],
                            tri_sb[:, (mrel - 4) * 512 + ia:(mrel - 4) * 512 + ib],
                            OP.mult)
                    first, last = (mi == 0), (mrel == 11)
                    nc.tensor.matmul(dn_ps[:, ia:ib], ones_sb[:], pt[:, ia:ib],
                                     start=first, stop=last, skip_group_check=True)
                    for dh in range(2):
                        nc.tensor.matmul(y_ps[dh][:, ia:ib],
                                         vtc[:, jh, dh * 128:(dh + 1) * 128],
                                         pt[:, ia:ib], start=first, stop=last,
                                         skip_group_check=True)
                    if oci < len(oc) and 5 <= mi < 12:
                        ms, dm, dq = oc[oci]
                        emit_oproj_chunk(a_prev, yt_prev, ms, dm, dq)
                        oci += 1
                rc = spool.tile([128, 512], f32, tag="rc")
                nc.vector.reciprocal_approx_fast(rc[:], dn_ps[:])
                for dh in range(2):
                    nc.vector.tensor_tensor(yt_sb[:, 2 * h + dh, :],
                                            y_ps[dh][:], rc[:], OP.mult)

            while oci < len(oc):
                ms, dm, dq = oc[oci]
                emit_oproj_chunk(a_prev, yt_prev, ms, dm, dq)
                oci += 1
            yt_prev, a_prev = yt_sb, a

        emit_oproj(a_prev, yt_prev)

    nc.compile()
    _cache["nc"] = nc
    return nc


def _run(inputs, trace=False):
    from concourse.bass_utils import run_bass_kernel_spmd

    nc = _build_program()
    in_maps = _host_prep(**inputs)
    res = run_bass_kernel_spmd(nc, in_maps, core_ids=list(range(8)), trace=trace)
    full = np.zeros((T, DM), np.float32)
    for g in range(NG):
        for s in range(NS):
            full[s * 2048:(s + 1) * 2048] += np.asarray(
                res.results[g * 2 + s]["o_part"], dtype=np.float32)
    return full.reshape(1, T, DM), res


def kernel(**inputs):
    return _run(inputs, trace=False)[0]
